# revision 27
# baseline (speedup 1.0000x reference)
"""FAGCN (4-layer FAConv + lin1/lin2 + log_softmax) on 8 Trainium2 cores.

v2 strategy (graph/data parallel, nodes sharded across 8 cores):
- Per layer the h-table is AllGathered as bf16 rows of 256B:
  [dinv_j*h_j (64 bf16) | al_j (1 f32, bytes 128:132) | zero pad].
  dinv_src is folded into the table row; dinv_dst is applied once per dst
  tile in the epilogue, so no per-slot norm multiply is needed.
- h[src] rows are fetched per edge with dma_gather (256B elems).  int16
  indices cover the 50176-row table via FOUR overlapping 32768-row
  windows; per-tile LP-optimal caps + fill-extremes assignment minimize
  slot padding (1016 columns vs 850 ideal).
- Gathers run in normal (non-prepare) mode: prepare_only+trigger_dma was
  measured to save nothing (Q7 desc-gen rate is identical) and produces
  NaN on hardware (the trigger's deferred RAW on the collective output
  does not hold).  The kernel is bound by Q7 SWDGE descriptor generation
  at ~7-8 ns/index.
- Pad slots point at pad-node rows (dinv=0 -> all-zero h), so they
  contribute nothing; no mask arrays.
- Coefficient: cf = tanh(al_src + ar_dst) via ACT (bias = per-partition
  ar), one op per (tile,window); messages scaled with ONE broadcast
  tensor_tensor per (chunk,window) (cf.to_broadcast), and segment-summed
  via per-slot identity matmuls into PSUM.  h_new = dinv*acc + EPS*raw.
- Phase 0 (relu(x@W1.T+b1)) runs in bf16 (x pre-cast on host).
"""
import numpy as np
from dataclasses import dataclass

import concourse.bass as bass
import concourse.bacc as bacc
import concourse.tile as tile
import concourse.mybir as mybir
from concourse import bass_utils
from concourse.masks import make_identity

F32 = mybir.dt.float32
BF16 = mybir.dt.bfloat16
I16 = mybir.dt.int16
AF = mybir.ActivationFunctionType
OP = mybir.AluOpType


@dataclass
class Cfg:
    N: int = 50000
    E: int = 800000
    F: int = 512
    H: int = 64
    C: int = 40
    L: int = 4
    EPS: float = 0.2
    M: int = 8           # cores
    CHUNK_COLS: int = 24  # max slot columns per window per gather chunk
    WINDOW: int = 16384  # dma_gather window size (int16 index limit 32768)
    SCRATCH: int = 32768  # SWDGE descriptor carveout (descs)
    NWA: int = 3         # windows over the first AllGather half
    NWB: int = 3         # windows over the second AllGather half
    NQ: int = 4          # SWDGE queues (desc-gen runs on DSP pair 2q,2q+1)
    SPLIT1: int = 25     # tiles in the first AllGather half

    @property
    def NSH(self):
        return self.N // self.M

    @property
    def TPC(self):
        return (self.NSH + 127) // 128

    @property
    def NSHP(self):
        return self.TPC * 128

    @property
    def RF(self):
        return self.NSHP * self.M

    @property
    def S1R(self):
        return self.SPLIT1 * 128

    @property
    def S2R(self):
        return self.NSHP - self.S1R

    @property
    def NW(self):
        return self.NWA + self.NWB

    @property
    def BOUND(self):
        return self.M * self.S1R

    @property
    def wgroups(self):
        """Per AllGather half: (bases, row_lo, row_hi).  Windows never cross
        the half boundary so each gather reads exactly one collective's
        output tile."""
        a_span = self.BOUND - self.WINDOW
        aa = tuple(round(a_span * w / (self.NWA - 1)) for w in range(self.NWA))
        b_span = (self.RF - self.BOUND) - self.WINDOW
        bb = tuple(self.BOUND + round(b_span * w / (self.NWB - 1))
                   for w in range(self.NWB))
        return ((aa, 0, self.BOUND), (bb, self.BOUND, self.RF))

    @property
    def wbases(self):
        return tuple(b for (bs, _, _) in self.wgroups for b in bs)

    @property
    def rank_of_pos(self):
        """Rank of the j-th real node of a core: rank S1R-1 is reserved as a
        zero pad row so the first half has one (ranks >= NSH pad the second
        half)."""
        return np.concatenate([np.arange(self.S1R - 1),
                               np.arange(self.S1R, self.NSH + 1)])

    def grow2(self, k, r):
        """Global table row of (core k, local rank r): half-major layout.
        Ranks < S1R live in the first AllGather half (rows [0, M*S1R)),
        the rest in the second half — so each half is one collective whose
        per-core contributions land contiguously."""
        r = np.asarray(r)
        return np.where(r < self.S1R, k * self.S1R + r,
                        self.M * self.S1R + k * self.S2R + (r - self.S1R))


def _rank_in_group(keys):
    """For a 1-D int key array, the 0-based rank of each element within its
    key group, counting in array order."""
    order = np.argsort(keys, kind="stable")
    sk = keys[order]
    first = np.concatenate([[0], np.flatnonzero(sk[1:] != sk[:-1]) + 1])
    starts = np.repeat(first, np.diff(np.concatenate([first, [len(sk)]])))
    rank = np.arange(len(sk)) - starts
    out = np.empty(len(sk), np.int64)
    out[order] = rank
    return out


def _wrap16(lst16):
    a = lst16.reshape(-1, 16).T.copy()
    return np.tile(a, (8, 1)).astype(np.int16)


def host_prep(cfg: Cfg, x, edge_index, W1, b1, W2, b2, att_l, att_r):
    import os
    self_local = os.environ.get("KSELF", "1") == "1"
    N, M, NSH, NSHP, TPC = cfg.N, cfg.M, cfg.NSH, cfg.NSHP, cfg.TPC
    W, NW = cfg.WINDOW, cfg.NW
    bases = cfg.wbases
    src = np.asarray(edge_index[0], dtype=np.int64)
    dst = np.asarray(edge_index[1], dtype=np.int64)
    loop = np.arange(N, dtype=np.int64)
    rows = np.concatenate([src, loop])
    cols = np.concatenate([dst, loop])
    deg = np.bincount(cols, minlength=N).astype(np.float32)
    dinv_n = (1.0 / np.sqrt(np.maximum(deg, 1.0))).astype(np.float32)
    nself = np.bincount(cols[rows == cols], minlength=N).astype(np.float32)
    if self_local:
        keep = rows != cols
        rows, cols = rows[keep], cols[keep]

    core_of = cols // NSH
    orders, inv_orders = [], []
    for k in range(M):
        degl = np.bincount(cols[core_of == k] - k * NSH, minlength=NSH)
        order = np.argsort(-degl, kind="stable")
        inv = np.empty(NSH, dtype=np.int64)
        inv[order] = np.arange(NSH)
        orders.append(order)
        inv_orders.append(inv)
    grow_map = np.empty(N, dtype=np.int64)
    for k in range(M):
        grow_map[k * NSH:(k + 1) * NSH] = cfg.grow2(k, inv_orders[k])

    # ---- pass A: per-core class counts; per-tile LP caps
    # Window eligibility of a row is a prefix [0..k] or suffix [j..NW-1]
    # of the window list.  Classes: 0..NW-1 = prefix [0..cls];
    # NW+j-1 = suffix [j..NW-1] (j>=1).
    t_of = np.arange(NSHP) // 128
    MP = [np.zeros(TPC, np.int64) for _ in range(NW)]
    MS = [np.zeros(TPC, np.int64) for _ in range(NW)]
    MD = np.zeros(TPC, np.int64)
    percore = []
    for k in range(M):
        m = core_of == k
        es = rows[m]
        rk = inv_orders[k][cols[m] - k * NSH]
        g = grow_map[es]
        memb = np.stack([(g >= bases[w]) & (g < bases[w] + W)
                         for w in range(NW)])
        wlo = memb.argmax(axis=0)
        whi = NW - 1 - memb[::-1].argmax(axis=0)
        cls = np.where(wlo == 0, whi, NW + wlo - 1).astype(np.int64)
        cnt = np.zeros((NSHP, 2 * NW - 1), np.int64)
        np.add.at(cnt, (rk, cls), 1)
        for b in range(NW):
            pc = np.bincount(rk[(wlo == 0) & (whi <= b)], minlength=NSHP)
            np.maximum.at(MP[b], t_of, pc)
        for a in range(1, NW):
            sc = np.bincount(rk[wlo >= a], minlength=NSHP)
            np.maximum.at(MS[a], t_of, sc)
        np.maximum.at(MD, t_of, cnt.sum(axis=1))
        percore.append((es, rk, g, cls, cnt))
    C = [None] * NW
    C[0] = MP[0].copy()
    acc = C[0].copy()
    for w in range(1, NW - 1):
        C[w] = np.maximum(0, MP[w] - acc)
        acc += C[w]
    last = MS[NW - 1].copy()
    for a in range(NW - 2, 0, -1):
        partial = sum(C[w] for w in range(a, NW - 1))
        last = np.maximum(last, MS[a] - partial)
    last = np.maximum(last, MD - acc)
    C[NW - 1] = last
    caps = np.stack(C)

    # per-core edge->window assignment honoring the caps: process classes
    # tightest-first (interleaving prefixes and suffixes by interval size);
    # prefixes fill low windows first, suffixes fill high windows first.
    for ki in range(M):
        es, rk, g, cls, cnt = percore[ki]
        loads = [np.zeros(NSHP, np.int64) for _ in range(NW)]
        fills = {}  # cls -> [(w, per-node count)] in fill order
        for sz in range(1, NW + 1):
            todo = [(sz - 1, list(range(0, sz)))]          # prefix [0, sz-1]
            j = NW - sz
            if j >= 1:                                      # suffix [j, NW-1]
                todo.append((NW + j - 1, list(range(NW - 1, j - 1, -1))))
            for c, worder in todo:
                rem = cnt[:, c].copy()
                fl = []
                for w in worder:
                    a = np.clip(caps[w][t_of] - loads[w], 0, rem)
                    loads[w] += a
                    rem -= a
                    fl.append((w, a))
                assert (rem == 0).all(), (ki, c)
                fills[c] = fl
        for w in range(NW):
            assert (loads[w] <= caps[w][t_of]).all()
        rc = _rank_in_group(rk * 16 + cls)  # rank within (node, class)
        win = np.full(len(es), -1, dtype=np.int64)
        for c, fl in fills.items():
            s = cls == c
            if not s.any():
                continue
            rcs, rks = rc[s], rk[s]
            res = np.full(rcs.shape, -1, dtype=np.int64)
            acc_th = np.zeros(NSHP, np.int64)
            for (w, a) in fl:
                acc_th = acc_th + a
                pick = (res == -1) & (rcs < acc_th[rks])
                res[pick] = w
            assert (res >= 0).all()
            win[s] = res
        assert (win >= 0).all()
        percore[ki] = (es, rk, g, win)
    offs = [np.zeros(TPC + 1, dtype=np.int64) for _ in range(NW)]
    for w in range(NW):
        np.cumsum(caps[w], out=offs[w][1:])
    Tw = [int(offs[w][-1]) for w in range(NW)]

    # per-window pad rows (dinv=0 -> zero h): some core's rank-NSH row that
    # falls inside the window.
    padrow = []
    for w in range(NW):
        for k in range(M):
            pr = int(cfg.grow2(k, np.int64(NSH))) - bases[w]
            if 0 <= pr < W:
                padrow.append(pr)
                break
        else:
            raise AssertionError(f"no pad row for window {w}")

    # ---- pass B: idx arrays
    in_maps = []
    for k in range(M):
        es, rk, g, win = percore[k]
        idxs = []
        for w in range(NW):
            m = win == w
            rkw, gw = rk[m], g[m]
            rcw = _rank_in_group(rkw)
            col = offs[w][rkw // 128] + rcw
            p = rkw % 128
            flat = np.full(Tw[w] * 128, padrow[w], dtype=np.int64)
            flat[col * 128 + p] = gw - bases[w]
            assert flat.size == 0 or (flat.min() >= 0 and flat.max() < W)
            idxs.append(_wrap16(flat.astype(np.int16)))

        import ml_dtypes
        xk = np.zeros((cfg.F, NSHP), dtype=np.float32)
        xk[:, :NSH] = np.asarray(x[k * NSH:(k + 1) * NSH], np.float32)[orders[k]].T
        dv = np.zeros((128, TPC), dtype=np.float32)
        dvk = dinv_n[k * NSH:(k + 1) * NSH][orders[k]]
        rr = np.arange(NSH)
        dv[rr % 128, rr // 128] = dvk  # rank r -> (p=r%128, t=r//128)
        sw = np.zeros((128, TPC), dtype=np.float32)
        swk = nself[k * NSH:(k + 1) * NSH][orders[k]]
        sw[rr % 128, rr // 128] = swk

        im = {
            "xT": np.ascontiguousarray(xk).astype(ml_dtypes.bfloat16),
            "W1T": np.ascontiguousarray(np.asarray(W1, np.float32).T),
            "b1": np.asarray(b1, np.float32).reshape(1, cfg.H),
            "W2T": np.ascontiguousarray(np.asarray(W2, np.float32).T),
            "b2": np.asarray(b2, np.float32).reshape(1, cfg.C),
            "attl": np.asarray(att_l, np.float32).reshape(1, -1),
            "attr": np.asarray(att_r, np.float32).reshape(1, -1),
            "dinv": dv,
            "selfw": sw,
        }
        for w in range(NW):
            im[f"idx{w}"] = idxs[w]
        in_maps.append(im)
    return in_maps, orders, [caps[w].tolist() for w in range(cfg.NW)]


def build_nc(cfg: Cfg, CAPS, use_prepare=True):
    import os
    al_f32 = os.environ.get("KALF32", "1") == "1"
    self_local = os.environ.get("KSELF", "1") == "1"
    caps = [np.asarray(v, dtype=np.int64) for v in CAPS]
    TPC, H, C, L = cfg.TPC, cfg.H, cfg.C, cfg.L
    NW = cfg.NW
    offs = [np.zeros(TPC + 1, dtype=np.int64) for _ in range(NW)]
    for w in range(NW):
        np.cumsum(caps[w], out=offs[w][1:])
    Tw = [int(offs[w][-1]) for w in range(NW)]
    NSLC = cfg.F // 128

    nc = bacc.Bacc("TRN2", target_bir_lowering=False, debug=False,
                   num_devices=cfg.M, dynamic_dma_scratch_size=cfg.SCRATCH,
                   num_swdge_queues=cfg.NQ)
    xT_h = nc.dram_tensor("xT", [cfg.F, cfg.NSHP], BF16, kind="ExternalInput")
    W1T_h = nc.dram_tensor("W1T", [cfg.F, H], F32, kind="ExternalInput")
    b1_h = nc.dram_tensor("b1", [1, H], F32, kind="ExternalInput")
    W2T_h = nc.dram_tensor("W2T", [H, C], F32, kind="ExternalInput")
    b2_h = nc.dram_tensor("b2", [1, C], F32, kind="ExternalInput")
    attl_h = nc.dram_tensor("attl", [1, L * H], F32, kind="ExternalInput")
    attr_h = nc.dram_tensor("attr", [1, L * H], F32, kind="ExternalInput")
    dinv_h = nc.dram_tensor("dinv", [128, TPC], F32, kind="ExternalInput")
    selfw_h = nc.dram_tensor("selfw", [128, TPC], F32, kind="ExternalInput")
    idx_h = [nc.dram_tensor(f"idx{w}", [128, 8 * Tw[w]], I16,
                            kind="ExternalInput") for w in range(NW)]
    out_h = nc.dram_tensor("out", [cfg.NSHP, C], F32, kind="ExternalOutput")

    # chunks: consecutive tiles with every window span <= CHUNK_COLS
    chunks = []
    t0 = 0
    for t in range(TPC + 1):
        if t == TPC or (t > t0 and any(
                offs[w][t] - offs[w][t0] + caps[w][t] > cfg.CHUNK_COLS
                for w in range(NW))):
            if t > t0:
                chunks.append((t0, t))
            t0 = t
    order = [(w, t0, t1) for (t0, t1) in chunks for w in range(NW)
             if offs[w][t1] > offs[w][t0]]

    with tile.TileContext(nc) as tc:
        dma_sem = nc.alloc_semaphore("swdge_dma")
        with tc.tile_pool(name="dram", bufs=2, space="DRAM") as dram, \
             tc.tile_pool(name="pers", bufs=1) as pers, \
             tc.tile_pool(name="gpool", bufs=2) as gpool, \
             tc.tile_pool(name="cpool", bufs=2) as cpool, \
             tc.tile_pool(name="spool", bufs=2) as spool, \
             tc.tile_pool(name="apsum", bufs=2, space="PSUM") as apsum, \
             tc.tile_pool(name="bpsum", bufs=2, space="PSUM") as bpsum:

            onesf = pers.tile([1, 128], F32)
            nc.vector.memset(onesf[:], 1.0)
            onesb = pers.tile([1, 128], BF16)
            nc.vector.memset(onesb[:], 1.0)
            ident = pers.tile([128, 128], F32)
            make_identity(nc, ident[:])
            identb = pers.tile([128, 128], BF16)
            nc.vector.tensor_copy(identb[:], ident[:])
            b1s = pers.tile([1, H], BF16)
            nc.gpsimd.dma_start(b1s[:], b1_h[:])
            b2s = pers.tile([1, C], F32)
            nc.sync.dma_start(b2s[:], b2_h[:])
            W2Ts = pers.tile([H, C], F32)
            nc.sync.dma_start(W2Ts[:], W2T_h[:])
            W1Ts = pers.tile([128, NSLC, H], BF16)
            nc.gpsimd.dma_start(W1Ts[:], W1T_h[:].rearrange("(s p) h -> p s h", p=128))
            attls = pers.tile([1, L * H], F32)
            nc.sync.dma_start(attls[:], attl_h[:])
            attrs = pers.tile([1, L * H], F32)
            nc.sync.dma_start(attrs[:], attr_h[:])
            dinv = pers.tile([128, TPC], F32)
            nc.sync.dma_start(dinv[:], dinv_h[:])
            selfw = pers.tile([128, TPC], F32)
            nc.sync.dma_start(selfw[:], selfw_h[:])
            idxs = []
            for w in range(NW):
                it = pers.tile([128, 8 * Tw[w]], I16, tag=f"idxt{w}")
                nc.sync.dma_start(it[:], idx_h[w][:])
                idxs.append(it)

            attbc = pers.tile([128, 2 * L, H], F32)
            for l in range(L):
                for j, srcrow in enumerate((attls, attrs)):
                    bc = bpsum.tile([128, H], F32, tag="bc")
                    nc.tensor.matmul(bc[:], lhsT=onesf[:],
                                     rhs=srcrow[0:1, l * H:(l + 1) * H],
                                     start=True, stop=True)
                    nc.vector.tensor_copy(attbc[:, 2 * l + j, :], bc[:])

            stage = pers.tile([128, TPC, H], F32)
            raw = pers.tile([128, TPC, H], F32)
            ar_sb = pers.tile([128, TPC], F32)
            tstage = pers.tile([128, TPC, 128], BF16)
            nc.vector.memset(tstage[:, :, 66:], 0.0)
            tstage_f32 = tstage[:].bitcast(F32)  # [128, TPC, 64]
            outs = pers.tile([128, TPC, C], F32)
            mx_all = pers.tile([128, TPC], F32)
            se_all = pers.tile([128, TPC], F32)
            lse_all = pers.tile([128, TPC], F32)

            alcol = pers.tile([128, TPC], F32)

            def table_entries(t, l):
                """After stage[:, t] holds h_l: write tstage row + al/ar."""
                nc.scalar.activation(
                    tstage[:, t, 0:H], stage[:, t, :], AF.Copy,
                    scale=dinv[:, t:t + 1])
                scr = cpool.tile([128, H], F32, tag="scr")
                al_dst = (tstage_f32[:, t, 32:33] if al_f32
                          else alcol[:, t:t + 1])
                nc.vector.tensor_tensor(
                    out=scr[:], in0=stage[:, t, :],
                    in1=attbc[:, 2 * l, :], op=OP.mult)
                nc.vector.tensor_reduce(
                    out=al_dst, in_=scr[:], axis=mybir.AxisListType.X,
                    op=OP.add)
                if not al_f32:
                    nc.vector.tensor_copy(tstage[:, t, 64:65],
                                          alcol[:, t:t + 1])
                scr2 = cpool.tile([128, H], F32, tag="scr")
                nc.vector.tensor_tensor(
                    out=scr2[:], in0=stage[:, t, :],
                    in1=attbc[:, 2 * l + 1, :], op=OP.mult)
                nc.vector.tensor_reduce(
                    out=ar_sb[:, t:t + 1], in_=scr2[:],
                    axis=mybir.AxisListType.X, op=OP.add)

            # ---- split AllGather plumbing (half-major table layout)
            S1T = cfg.SPLIT1
            RG = [list(range(cfg.M))]
            tbl_fulls = {}

            def issue_tblA(lnext):
                """First-half AllGather: issued as soon as tstage tiles
                [0, S1T) for layer lnext are written — overlaps the tail of
                the previous layer's chunk processing."""
                tf = dram.tile([cfg.RF, 128], BF16, tag="tbl_full")
                tbl_fulls[lnext] = tf
                ta = dram.tile([cfg.S1R, 128], BF16, tag="tbl_inA")
                nc.sync.dma_start(
                    ta[:].rearrange("(t p) e -> p t e", p=128),
                    tstage[:, 0:S1T, :])
                nc.gpsimd.collective_compute(
                    "AllGather", OP.bypass, replica_groups=RG,
                    ins=[ta[:].opt()],
                    outs=[tbl_fulls[lnext][0:cfg.M * cfg.S1R, :].opt()])

            def issue_tblB(lnext):
                tb = dram.tile([cfg.S2R, 128], BF16, tag="tbl_inB")
                nc.sync.dma_start(
                    tb[:].rearrange("(t p) e -> p t e", p=128),
                    tstage[:, S1T:, :])
                nc.gpsimd.collective_compute(
                    "AllGather", OP.bypass, replica_groups=RG,
                    ins=[tb[:].opt()],
                    outs=[tbl_fulls[lnext][cfg.M * cfg.S1R:, :].opt()])

            # ---- phase 0: h0 = relu(x @ W1.T + b1); table entries for l=0
            XG = 4
            with tc.tile_pool(name="xpool", bufs=2) as xpool:
                for g0 in range(0, TPC, XG):
                    g1 = min(g0 + XG, TPC)
                    cw = (g1 - g0) * 128
                    xt = xpool.tile([128, NSLC, cw], BF16, tag="xt")
                    nc.sync.dma_start(
                        xt[:, :, :],
                        xT_h[:, g0 * 128:g1 * 128].rearrange(
                            "(s p) c -> p s c", p=128))
                    for t in range(g0, g1):
                        lc = (t - g0) * 128
                        acc = apsum.tile([128, H], F32, tag="acc")
                        nc.tensor.matmul(acc[:], lhsT=onesb[:], rhs=b1s[:],
                                         start=True, stop=False)
                        for s in range(NSLC):
                            nc.tensor.matmul(acc[:], lhsT=xt[:, s, lc:lc + 128],
                                             rhs=W1Ts[:, s, :],
                                             start=False, stop=(s == NSLC - 1))
                        nc.scalar.activation(stage[:, t, :], acc[:], AF.Relu)
                        nc.vector.tensor_copy(raw[:, t, :], stage[:, t, :])
                        table_entries(t, 0)
                        if t == S1T - 1:
                            issue_tblA(0)

            # ---- layers
            qload = [0] * cfg.NQ  # greedy per-queue index balance
            issue_tblB(0)
            for l in range(L):
                tbl_full = tbl_fulls.pop(l)

                for (ct0, ct1) in chunks:
                    spans = []
                    gts = []
                    for w in range(NW):
                        c0, c1 = int(offs[w][ct0]), int(offs[w][ct1])
                        spans.append((c0, c1))
                        if c1 == c0:
                            gts.append(None)
                            continue
                        gw = gpool.tile([128, cfg.CHUNK_COLS, 128], BF16,
                                        tag=f"g{w}")
                        lo = cfg.wbases[w]
                        q = min(range(cfg.NQ), key=lambda i: qload[i])
                        qload[q] += c1 - c0
                        kw = dict(
                            out_ap=gw[:, :c1 - c0, :],
                            in_ap=tbl_full[lo:lo + cfg.WINDOW, :],
                            idxs_ap=idxs[w][:, 8 * c0:8 * c1],
                            num_idxs=128 * (c1 - c0),
                            num_idxs_reg=128 * (c1 - c0),
                            elem_size=128, single_packet=False,
                            queue_num=q)
                        if use_prepare:
                            nc.gpsimd.dma_gather(prepare_only=True,
                                                 sem=dma_sem, **kw)
                        else:
                            nc.gpsimd.dma_gather(**kw)
                        gts.append(gw)
                    if use_prepare:
                        nc.gpsimd.trigger_dma(count=None)

                    # coefficients + messages per window
                    msgs = []
                    for w in range(NW):
                        c0, c1 = spans[w]
                        if c1 == c0:
                            msgs.append(None)
                            continue
                        gw = gts[w]
                        span = c1 - c0
                        cf = cpool.tile([128, cfg.CHUNK_COLS], BF16, tag=f"cf{w}")
                        gw_f32 = gw[:, :span, :].bitcast(F32)
                        for t in range(ct0, ct1):
                            nn = int(caps[w][t])
                            if nn == 0:
                                continue
                            lc = int(offs[w][t]) - c0
                            al_src = (gw_f32[:, lc:lc + nn, 32:33] if al_f32
                                      else gw[:, lc:lc + nn, 64:65])
                            nc.scalar.activation(
                                cf[:, lc:lc + nn], al_src,
                                AF.Tanh, bias=ar_sb[:, t:t + 1])
                        msg = cpool.tile([128, cfg.CHUNK_COLS, H], BF16,
                                         tag=f"msg{w}")
                        nc.vector.tensor_tensor(
                            out=msg[:, :span, :],
                            in0=gw[:, :span, 0:H],
                            in1=cf[:, :span].to_broadcast((128, span, H)),
                            op=OP.mult)
                        msgs.append(msg)

                    for t in range(ct0, ct1):
                        nblk = sum(int(caps[w][t]) for w in range(NW))
                        acc = apsum.tile([128, H], F32, tag="acc")
                        if self_local:
                            # self message: selfw*dinv*tanh(al+ar)*h, from the
                            # local table row (tstage = dinv*h, al)
                            cfs = cpool.tile([128, 1], F32, tag="cfs")
                            al_self = (tstage_f32[:, t, 32:33] if al_f32
                                       else tstage[:, t, 64:65])
                            nc.scalar.activation(cfs[:], al_self, AF.Tanh,
                                                 bias=ar_sb[:, t:t + 1])
                            msl = cpool.tile([128, H], BF16, tag="msgself")
                            nc.vector.tensor_scalar(
                                out=msl[:], in0=tstage[:, t, 0:H],
                                scalar1=cfs[:, 0:1],
                                scalar2=selfw[:, t:t + 1],
                                op0=OP.mult, op1=OP.mult)
                            nc.tensor.matmul(acc[:], lhsT=identb[:],
                                             rhs=msl[:], start=True,
                                             stop=(nblk == 0))
                            bi = 1
                            nblk += 1
                        else:
                            bi = 0
                        for w in range(NW):
                            nn = int(caps[w][t])
                            if nn == 0:
                                continue
                            lc = int(offs[w][t]) - spans[w][0]
                            for b in range(nn):
                                nc.tensor.matmul(
                                    acc[:], lhsT=identb[:],
                                    rhs=msgs[w][:, lc + b, :],
                                    start=(bi == 0), stop=(bi == nblk - 1))
                                bi += 1
                        tmp = cpool.tile([128, H], F32, tag="tmp")
                        nc.scalar.activation(
                            tmp[:], acc[:], AF.Copy,
                            scale=dinv[:, t:t + 1])
                        nc.vector.scalar_tensor_tensor(
                            out=stage[:, t, :], in0=raw[:, t, :],
                            scalar=cfg.EPS, in1=tmp[:],
                            op0=OP.mult, op1=OP.add)
                        if l < L - 1:
                            table_entries(t, l + 1)
                            if t == S1T - 1:
                                issue_tblA(l + 1)
                if l < L - 1:
                    issue_tblB(l + 1)

            # ---- logits + log_softmax
            for t in range(TPC):
                tr = bpsum.tile([H, 128], F32, tag="tr")
                nc.tensor.transpose(out=tr[:], in_=stage[:, t, :],
                                    identity=ident[:])
                htT = spool.tile([H, 128], F32, tag="htT")
                nc.vector.tensor_copy(htT[:], tr[:])
                lg = bpsum.tile([128, C], F32, tag="lg")
                nc.tensor.matmul(lg[:], lhsT=onesf[:], rhs=b2s[:],
                                 start=True, stop=False)
                nc.tensor.matmul(lg[:], lhsT=htT[:], rhs=W2Ts[:],
                                 start=False, stop=True)
                nc.vector.tensor_reduce(out=mx_all[:, t:t + 1], in_=lg[:],
                                        axis=mybir.AxisListType.X, op=OP.max,
                                        negate=True)
                scr40 = cpool.tile([128, C], F32, tag="scr40")
                nc.scalar.activation(scr40[:], lg[:], AF.Exp,
                                     bias=mx_all[:, t:t + 1],
                                     accum_out=se_all[:, t:t + 1])
                nc.vector.tensor_copy(outs[:, t, :], lg[:])
            nc.scalar.activation(lse_all[:], se_all[:], AF.Ln)
            for t in range(TPC):
                nc.vector.tensor_scalar(
                    out=outs[:, t, :], in0=outs[:, t, :],
                    scalar1=mx_all[:, t:t + 1], scalar2=lse_all[:, t:t + 1],
                    op0=OP.add, op1=OP.subtract)
            nc.sync.dma_start(out_h[:].rearrange("(t p) c -> p t c", p=128),
                              outs[:])
    nc.compile()
    return nc


def run(cfg: Cfg, inputs: dict, trace: bool = False, use_prepare: bool = None):
    if use_prepare is None:
        import os
        use_prepare = os.environ.get("KPREP", "0") == "1"
    in_maps, orders, CAPS = host_prep(cfg, **inputs)
    nc = build_nc(cfg, CAPS, use_prepare=use_prepare)
    res = bass_utils.run_bass_kernel_spmd(
        nc, in_maps, core_ids=list(range(cfg.M)), trace=trace)
    out = np.empty((cfg.N, cfg.C), dtype=np.float32)
    for k in range(cfg.M):
        out[k * cfg.NSH + orders[k]] = np.asarray(res.results[k]["out"],
                                                  np.float32)[:cfg.NSH]
    return out, res


def kernel(x, edge_index, W1, b1, W2, b2, att_l, att_r):
    cfg = Cfg()
    out, _ = run(cfg, dict(x=np.asarray(x, np.float32),
                           edge_index=np.asarray(edge_index),
                           W1=W1, b1=b1, W2=W2, b2=b2,
                           att_l=att_l, att_r=att_r))
    return out



# revision 38
# speedup vs baseline: 1.2290x; 1.2290x over previous
"""FAGCN (4-layer FAConv + lin1/lin2 + log_softmax) on 8 Trainium2 cores.

v2 strategy (graph/data parallel, nodes sharded across 8 cores):
- Per layer the h-table is AllGathered as bf16 rows of 256B:
  [dinv_j*h_j (64 bf16) | al_j (1 f32, bytes 128:132) | zero pad].
  dinv_src is folded into the table row; dinv_dst is applied once per dst
  tile in the epilogue, so no per-slot norm multiply is needed.
- h[src] rows are fetched per edge with dma_gather (256B elems).  int16
  indices cover the 50176-row table via FOUR overlapping 32768-row
  windows; per-tile LP-optimal caps + fill-extremes assignment minimize
  slot padding (1016 columns vs 850 ideal).
- Gathers run in normal (non-prepare) mode: prepare_only+trigger_dma was
  measured to save nothing (Q7 desc-gen rate is identical) and produces
  NaN on hardware (the trigger's deferred RAW on the collective output
  does not hold).  The kernel is bound by Q7 SWDGE descriptor generation
  at ~7-8 ns/index.
- Pad slots point at pad-node rows (dinv=0 -> all-zero h), so they
  contribute nothing; no mask arrays.
- Coefficient: cf = tanh(al_src + ar_dst) via ACT (bias = per-partition
  ar), one op per (tile,window); messages scaled with ONE broadcast
  tensor_tensor per (chunk,window) (cf.to_broadcast), and segment-summed
  via per-slot identity matmuls into PSUM.  h_new = dinv*acc + EPS*raw.
- Phase 0 (relu(x@W1.T+b1)) runs in bf16 (x pre-cast on host).
"""
import numpy as np
from dataclasses import dataclass

import concourse.bass as bass
import concourse.bacc as bacc
import concourse.tile as tile
import concourse.mybir as mybir
from concourse import bass_utils
from concourse.masks import make_identity

F32 = mybir.dt.float32
BF16 = mybir.dt.bfloat16
I16 = mybir.dt.int16
AF = mybir.ActivationFunctionType
OP = mybir.AluOpType


@dataclass
class Cfg:
    N: int = 50000
    E: int = 800000
    F: int = 512
    H: int = 64
    C: int = 40
    L: int = 4
    EPS: float = 0.2
    M: int = 8           # cores
    CHUNK_COLS: int = 24  # max slot columns per window per gather chunk
    WINDOW: int = 32768  # dma_gather window size (int16 index limit 32768)
    SCRATCH: int = 32768  # SWDGE descriptor carveout (descs)
    NWA: int = 5         # windows (over the whole table when NWB == 0)
    NWB: int = 0         # >0 would split windows per AllGather half
    NQ: int = 4          # SWDGE queues (desc-gen runs on DSP pair 2q,2q+1)
    SPLIT1: int = 25     # tiles in the first AllGather half

    @property
    def NSH(self):
        return self.N // self.M

    @property
    def TPC(self):
        return (self.NSH + 127) // 128

    @property
    def NSHP(self):
        return self.TPC * 128

    @property
    def RF(self):
        return self.NSHP * self.M

    @property
    def S1R(self):
        return self.SPLIT1 * 128

    @property
    def S2R(self):
        return self.NSHP - self.S1R

    @property
    def NW(self):
        return self.NWA + self.NWB

    @property
    def BOUND(self):
        return self.M * self.S1R

    @property
    def wgroups(self):
        """Window groups: (bases, row_lo, row_hi).  With NWB == 0, one group
        of NWA windows over the whole rank-major table."""
        if self.NWB == 0:
            span = self.RF - self.WINDOW
            bs = tuple(round(span * w / (self.NWA - 1))
                       for w in range(self.NWA))
            return ((bs, 0, self.RF),)
        a_span = self.BOUND - self.WINDOW
        aa = tuple(round(a_span * w / (self.NWA - 1)) for w in range(self.NWA))
        b_span = (self.RF - self.BOUND) - self.WINDOW
        bb = tuple(self.BOUND + round(b_span * w / (self.NWB - 1))
                   for w in range(self.NWB))
        return ((aa, 0, self.BOUND), (bb, self.BOUND, self.RF))

    @property
    def wbases(self):
        return tuple(b for (bs, _, _) in self.wgroups for b in bs)

    @property
    def rank_of_pos(self):
        """Rank of the j-th real node of a core.  With half-split collectives
        (NWB > 0) rank S1R-1 is reserved as a zero pad row for the first
        half (ranks >= NSH pad the second half)."""
        if self.NWB == 0:
            return np.arange(self.NSH)
        return np.concatenate([np.arange(self.S1R - 1),
                               np.arange(self.S1R, self.NSH + 1)])

    def grow2(self, k, r):
        """Global table row of (core k, local rank r)."""
        r = np.asarray(r)
        if self.NWB == 0:
            return k * self.NSHP + r
        return np.where(r < self.S1R, k * self.S1R + r,
                        self.M * self.S1R + k * self.S2R + (r - self.S1R))


def _rank_in_group(keys):
    """For a 1-D int key array, the 0-based rank of each element within its
    key group, counting in array order."""
    order = np.argsort(keys, kind="stable")
    sk = keys[order]
    first = np.concatenate([[0], np.flatnonzero(sk[1:] != sk[:-1]) + 1])
    starts = np.repeat(first, np.diff(np.concatenate([first, [len(sk)]])))
    rank = np.arange(len(sk)) - starts
    out = np.empty(len(sk), np.int64)
    out[order] = rank
    return out


def _wrap16(lst16):
    a = lst16.reshape(-1, 16).T.copy()
    return np.tile(a, (8, 1)).astype(np.int16)


def _assign_group(cfg, bases, edges, t_of):
    """LP caps + per-core edge->window assignment for one window group.
    `bases`: window bases (all windows span [base, base+WINDOW), covering a
    half that no window crosses).  `edges`: per-core (es, rk, g) with g
    already inside this group's row range.  Returns (caps [NWg, TPC],
    win per-core list).  Window eligibility of a row is a prefix [0..k] or
    suffix [j..NWg-1] of the group's window list (requires
    bases[-1] - bases[0] <= WINDOW)."""
    W = cfg.WINDOW
    NW = len(bases)
    NSHP, TPC, M = cfg.NSHP, cfg.TPC, cfg.M
    MP = [np.zeros(TPC, np.int64) for _ in range(NW)]
    MS = [np.zeros(TPC, np.int64) for _ in range(NW)]
    MD = np.zeros(TPC, np.int64)
    percore = []
    for k in range(M):
        es, rk, g = edges[k]
        memb = np.stack([(g >= bases[w]) & (g < bases[w] + W)
                         for w in range(NW)])
        wlo = memb.argmax(axis=0)
        whi = NW - 1 - memb[::-1].argmax(axis=0)
        cls = np.where(wlo == 0, whi, NW + wlo - 1).astype(np.int64)
        cnt = np.zeros((NSHP, 2 * NW - 1), np.int64)
        np.add.at(cnt, (rk, cls), 1)
        for b in range(NW):
            pc = np.bincount(rk[(wlo == 0) & (whi <= b)], minlength=NSHP)
            np.maximum.at(MP[b], t_of, pc)
        for a in range(1, NW):
            sc = np.bincount(rk[wlo >= a], minlength=NSHP)
            np.maximum.at(MS[a], t_of, sc)
        np.maximum.at(MD, t_of, cnt.sum(axis=1))
        percore.append((es, rk, g, cls, cnt))
    C = [None] * NW
    C[0] = MP[0].copy()
    acc = C[0].copy()
    for w in range(1, NW - 1):
        C[w] = np.maximum(0, MP[w] - acc)
        acc += C[w]
    last = MS[NW - 1].copy() if NW > 1 else np.zeros(TPC, np.int64)
    for a in range(NW - 2, 0, -1):
        partial = sum(C[w] for w in range(a, NW - 1))
        last = np.maximum(last, MS[a] - partial)
    last = np.maximum(last, MD - acc)
    if NW > 1:
        C[NW - 1] = last
    else:
        C[0] = np.maximum(C[0], MD)
    caps = np.stack(C)

    # per-core edge->window assignment honoring the caps: process classes
    # tightest-first (interleaving prefixes and suffixes by interval size);
    # prefixes fill low windows first, suffixes fill high windows first.
    wins = []
    for ki in range(M):
        es, rk, g, cls, cnt = percore[ki]
        loads = [np.zeros(NSHP, np.int64) for _ in range(NW)]
        fills = {}  # cls -> [(w, per-node count)] in fill order
        for sz in range(1, NW + 1):
            todo = [(sz - 1, list(range(0, sz)))]          # prefix [0, sz-1]
            j = NW - sz
            if j >= 1:                                      # suffix [j, NW-1]
                todo.append((NW + j - 1, list(range(NW - 1, j - 1, -1))))
            for c, worder in todo:
                rem = cnt[:, c].copy()
                fl = []
                for w in worder:
                    a = np.clip(caps[w][t_of] - loads[w], 0, rem)
                    loads[w] += a
                    rem -= a
                    fl.append((w, a))
                assert (rem == 0).all(), (ki, c)
                fills[c] = fl
        for w in range(NW):
            assert (loads[w] <= caps[w][t_of]).all()
        rc = _rank_in_group(rk * 16 + cls)  # rank within (node, class)
        win = np.full(len(es), -1, dtype=np.int64)
        for c, fl in fills.items():
            s = cls == c
            if not s.any():
                continue
            rcs, rks = rc[s], rk[s]
            res = np.full(rcs.shape, -1, dtype=np.int64)
            acc_th = np.zeros(NSHP, np.int64)
            for (w, a) in fl:
                acc_th = acc_th + a
                pick = (res == -1) & (rcs < acc_th[rks])
                res[pick] = w
            assert (res >= 0).all()
            win[s] = res
        assert (win >= 0).all()
        wins.append(win)
    return caps, wins


def host_prep(cfg: Cfg, x, edge_index, W1, b1, W2, b2, att_l, att_r):
    import os
    self_local = os.environ.get("KSELF", "1") == "1"
    N, M, NSH, NSHP, TPC = cfg.N, cfg.M, cfg.NSH, cfg.NSHP, cfg.TPC
    W, NW = cfg.WINDOW, cfg.NW
    bases = cfg.wbases
    rop = cfg.rank_of_pos
    src = np.asarray(edge_index[0], dtype=np.int64)
    dst = np.asarray(edge_index[1], dtype=np.int64)
    loop = np.arange(N, dtype=np.int64)
    rows = np.concatenate([src, loop])
    cols = np.concatenate([dst, loop])
    deg = np.bincount(cols, minlength=N).astype(np.float32)
    dinv_n = (1.0 / np.sqrt(np.maximum(deg, 1.0))).astype(np.float32)
    nself = np.bincount(cols[rows == cols], minlength=N).astype(np.float32)
    if self_local:
        keep = rows != cols
        rows, cols = rows[keep], cols[keep]

    core_of = cols // NSH
    orders, inv_orders = [], []
    for k in range(M):
        degl = np.bincount(cols[core_of == k] - k * NSH, minlength=NSH)
        order = np.argsort(-degl, kind="stable")
        inv_pos = np.empty(NSH, dtype=np.int64)
        inv_pos[order] = np.arange(NSH)
        orders.append(order)
        inv_orders.append(rop[inv_pos])  # node -> rank (pad rank skipped)
    grow_map = np.empty(N, dtype=np.int64)
    for k in range(M):
        grow_map[k * NSH:(k + 1) * NSH] = cfg.grow2(k, inv_orders[k])

    # ---- pass A per window group: per-tile LP caps + window assignment
    t_of = np.arange(NSHP) // 128
    edges_all = []
    for k in range(M):
        m = core_of == k
        es = rows[m]
        rk = inv_orders[k][cols[m] - k * NSH]
        edges_all.append((es, rk, grow_map[es]))
    caps_l, wins_l, w0 = [], [[] for _ in range(M)], 0
    for (gb, glo, ghi) in cfg.wgroups:
        edges_g = []
        for k in range(M):
            es, rk, g = edges_all[k]
            m = (g >= glo) & (g < ghi)
            edges_g.append((es[m], rk[m], g[m]))
        caps_g, wins_g = _assign_group(cfg, gb, edges_g, t_of)
        caps_l.append(caps_g)
        for k in range(M):
            es, rk, g = edges_g[k]
            wins_l[k].append((es, rk, g, wins_g[k] + w0))
        w0 += len(gb)
    caps = np.concatenate(caps_l, axis=0)
    percore = []
    for k in range(M):
        es = np.concatenate([e for (e, _, _, _) in wins_l[k]])
        rk = np.concatenate([r for (_, r, _, _) in wins_l[k]])
        g = np.concatenate([gg for (_, _, gg, _) in wins_l[k]])
        win = np.concatenate([w for (_, _, _, w) in wins_l[k]])
        percore.append((es, rk, g, win))
    offs = [np.zeros(TPC + 1, dtype=np.int64) for _ in range(NW)]
    for w in range(NW):
        np.cumsum(caps[w], out=offs[w][1:])
    Tw = [int(offs[w][-1]) for w in range(NW)]

    # per-window pad rows (dinv=0 -> zero h): some core's pad-rank row that
    # falls inside the window.
    if cfg.NWB == 0:
        pads = list(range(NSH, NSHP))
    else:
        pads = [cfg.S1R - 1] + list(range(NSH + 1, NSHP))
    padrow = []
    for w in range(NW):
        hit = None
        for k in range(M):
            for pr0 in pads:
                pr = int(cfg.grow2(k, np.int64(pr0))) - bases[w]
                if 0 <= pr < W:
                    hit = pr
                    break
            if hit is not None:
                break
        if hit is None:
            raise AssertionError(f"no pad row for window {w}")
        padrow.append(hit)

    # ---- pass B: idx arrays
    in_maps = []
    for k in range(M):
        es, rk, g, win = percore[k]
        idxs = []
        for w in range(NW):
            m = win == w
            rkw, gw = rk[m], g[m]
            rcw = _rank_in_group(rkw)
            col = offs[w][rkw // 128] + rcw
            p = rkw % 128
            flat = np.full(Tw[w] * 128, padrow[w], dtype=np.int64)
            flat[col * 128 + p] = gw - bases[w]
            assert flat.size == 0 or (flat.min() >= 0 and flat.max() < W)
            idxs.append(_wrap16(flat.astype(np.int16)))

        import ml_dtypes
        xk = np.zeros((cfg.F, NSHP), dtype=np.float32)
        xk[:, rop] = np.asarray(x[k * NSH:(k + 1) * NSH], np.float32)[orders[k]].T
        dv = np.zeros((128, TPC), dtype=np.float32)
        dvk = dinv_n[k * NSH:(k + 1) * NSH][orders[k]]
        rr = rop
        dv[rr % 128, rr // 128] = dvk  # rank r -> (p=r%128, t=r//128)
        sw = np.zeros((128, TPC), dtype=np.float32)
        swk = nself[k * NSH:(k + 1) * NSH][orders[k]]
        sw[rr % 128, rr // 128] = swk

        im = {
            "xT": np.ascontiguousarray(xk).astype(ml_dtypes.bfloat16),
            "W1T": np.ascontiguousarray(np.asarray(W1, np.float32).T),
            "b1": np.asarray(b1, np.float32).reshape(1, cfg.H),
            "W2T": np.ascontiguousarray(np.asarray(W2, np.float32).T),
            "b2": np.asarray(b2, np.float32).reshape(1, cfg.C),
            "attl": np.asarray(att_l, np.float32).reshape(1, -1),
            "attr": np.asarray(att_r, np.float32).reshape(1, -1),
            "dinv": dv,
            "selfw": sw,
        }
        for w in range(NW):
            im[f"idx{w}"] = idxs[w]
        in_maps.append(im)
    return in_maps, orders, [caps[w].tolist() for w in range(cfg.NW)]


def build_nc(cfg: Cfg, CAPS, use_prepare=True):
    import os
    al_f32 = os.environ.get("KALF32", "1") == "1"
    self_local = os.environ.get("KSELF", "1") == "1"
    caps = [np.asarray(v, dtype=np.int64) for v in CAPS]
    TPC, H, C, L = cfg.TPC, cfg.H, cfg.C, cfg.L
    NW = cfg.NW
    offs = [np.zeros(TPC + 1, dtype=np.int64) for _ in range(NW)]
    for w in range(NW):
        np.cumsum(caps[w], out=offs[w][1:])
    Tw = [int(offs[w][-1]) for w in range(NW)]
    NSLC = cfg.F // 128

    nc = bacc.Bacc("TRN2", target_bir_lowering=False, debug=False,
                   num_devices=cfg.M, dynamic_dma_scratch_size=cfg.SCRATCH,
                   num_swdge_queues=cfg.NQ)
    xT_h = nc.dram_tensor("xT", [cfg.F, cfg.NSHP], BF16, kind="ExternalInput")
    W1T_h = nc.dram_tensor("W1T", [cfg.F, H], F32, kind="ExternalInput")
    b1_h = nc.dram_tensor("b1", [1, H], F32, kind="ExternalInput")
    W2T_h = nc.dram_tensor("W2T", [H, C], F32, kind="ExternalInput")
    b2_h = nc.dram_tensor("b2", [1, C], F32, kind="ExternalInput")
    attl_h = nc.dram_tensor("attl", [1, L * H], F32, kind="ExternalInput")
    attr_h = nc.dram_tensor("attr", [1, L * H], F32, kind="ExternalInput")
    dinv_h = nc.dram_tensor("dinv", [128, TPC], F32, kind="ExternalInput")
    selfw_h = nc.dram_tensor("selfw", [128, TPC], F32, kind="ExternalInput")
    idx_h = [nc.dram_tensor(f"idx{w}", [128, 8 * Tw[w]], I16,
                            kind="ExternalInput") for w in range(NW)]
    out_h = nc.dram_tensor("out", [cfg.NSHP, C], F32, kind="ExternalOutput")

    # chunks: consecutive tiles with every window span <= CHUNK_COLS
    chunks = []
    t0 = 0
    for t in range(TPC + 1):
        if t == TPC or (t > t0 and any(
                offs[w][t] - offs[w][t0] + caps[w][t] > cfg.CHUNK_COLS
                for w in range(NW))):
            if t > t0:
                chunks.append((t0, t))
            t0 = t
    order = [(w, t0, t1) for (t0, t1) in chunks for w in range(NW)
             if offs[w][t1] > offs[w][t0]]

    with tile.TileContext(nc) as tc:
        dma_sem = nc.alloc_semaphore("swdge_dma")
        with tc.tile_pool(name="dram", bufs=2, space="DRAM") as dram, \
             tc.tile_pool(name="pers", bufs=1) as pers, \
             tc.tile_pool(name="gpool", bufs=3) as gpool, \
             tc.tile_pool(name="cpool", bufs=2) as cpool, \
             tc.tile_pool(name="spool", bufs=2) as spool, \
             tc.tile_pool(name="apsum", bufs=2, space="PSUM") as apsum, \
             tc.tile_pool(name="bpsum", bufs=2, space="PSUM") as bpsum:

            onesf = pers.tile([1, 128], F32)
            nc.vector.memset(onesf[:], 1.0)
            onesb = pers.tile([1, 128], BF16)
            nc.vector.memset(onesb[:], 1.0)
            ident = pers.tile([128, 128], F32)
            make_identity(nc, ident[:])
            identb = pers.tile([128, 128], BF16)
            nc.vector.tensor_copy(identb[:], ident[:])
            b1s = pers.tile([1, H], BF16)
            nc.gpsimd.dma_start(b1s[:], b1_h[:])
            b2s = pers.tile([1, C], F32)
            nc.sync.dma_start(b2s[:], b2_h[:])
            W2Ts = pers.tile([H, C], F32)
            nc.sync.dma_start(W2Ts[:], W2T_h[:])
            W1Ts = pers.tile([128, NSLC, H], BF16)
            nc.gpsimd.dma_start(W1Ts[:], W1T_h[:].rearrange("(s p) h -> p s h", p=128))
            attls = pers.tile([1, L * H], F32)
            nc.sync.dma_start(attls[:], attl_h[:])
            attrs = pers.tile([1, L * H], F32)
            nc.sync.dma_start(attrs[:], attr_h[:])
            dinv = pers.tile([128, TPC], F32)
            nc.sync.dma_start(dinv[:], dinv_h[:])
            selfw = pers.tile([128, TPC], F32)
            nc.sync.dma_start(selfw[:], selfw_h[:])
            idxs = []
            for w in range(NW):
                it = pers.tile([128, 8 * Tw[w]], I16, tag=f"idxt{w}")
                nc.sync.dma_start(it[:], idx_h[w][:])
                idxs.append(it)

            attbc = pers.tile([128, 2 * L, H], F32)
            for l in range(L):
                for j, srcrow in enumerate((attls, attrs)):
                    bc = bpsum.tile([128, H], F32, tag="bc")
                    nc.tensor.matmul(bc[:], lhsT=onesf[:],
                                     rhs=srcrow[0:1, l * H:(l + 1) * H],
                                     start=True, stop=True)
                    nc.vector.tensor_copy(attbc[:, 2 * l + j, :], bc[:])

            stage = pers.tile([128, TPC, H], F32)
            raw = pers.tile([128, TPC, H], BF16)
            ar_sb = pers.tile([128, TPC], F32)
            tstage = pers.tile([128, TPC, 128], BF16)
            nc.vector.memset(tstage[:, :, 66:], 0.0)
            tstage_f32 = tstage[:].bitcast(F32)  # [128, TPC, 64]
            outs = pers.tile([128, TPC, C], F32)
            mx_all = pers.tile([128, TPC], F32)
            se_all = pers.tile([128, TPC], F32)
            lse_all = pers.tile([128, TPC], F32)

            alcol = pers.tile([128, TPC], F32)

            def table_entries(t, l):
                """After stage[:, t] holds h_l: write tstage row + al/ar."""
                nc.scalar.activation(
                    tstage[:, t, 0:H], stage[:, t, :], AF.Copy,
                    scale=dinv[:, t:t + 1])
                scr = cpool.tile([128, H], F32, tag="scr")
                al_dst = (tstage_f32[:, t, 32:33] if al_f32
                          else alcol[:, t:t + 1])
                nc.vector.tensor_tensor(
                    out=scr[:], in0=stage[:, t, :],
                    in1=attbc[:, 2 * l, :], op=OP.mult)
                nc.vector.tensor_reduce(
                    out=al_dst, in_=scr[:], axis=mybir.AxisListType.X,
                    op=OP.add)
                if not al_f32:
                    nc.vector.tensor_copy(tstage[:, t, 64:65],
                                          alcol[:, t:t + 1])
                scr2 = cpool.tile([128, H], F32, tag="scr")
                nc.vector.tensor_tensor(
                    out=scr2[:], in0=stage[:, t, :],
                    in1=attbc[:, 2 * l + 1, :], op=OP.mult)
                nc.vector.tensor_reduce(
                    out=ar_sb[:, t:t + 1], in_=scr2[:],
                    axis=mybir.AxisListType.X, op=OP.add)

            # ---- AllGather plumbing: one collective per layer, issued as
            # soon as the last table entry for that layer is written.
            RG = [list(range(cfg.M))]
            tbl_fulls = {}

            def issue_tbl(lnext):
                tf = dram.tile([cfg.RF, 128], BF16, tag="tbl_full",
                               addr_space="Shared")
                tbl_fulls[lnext] = tf
                ti = dram.tile([cfg.NSHP, 128], BF16, tag="tbl_in")
                nc.sync.dma_start(
                    ti[:].rearrange("(t p) e -> p t e", p=128), tstage[:])
                nc.gpsimd.collective_compute(
                    "AllGather", OP.bypass, replica_groups=RG,
                    ins=[ti[:].opt()], outs=[tf[:].opt()])

            # ---- phase 0: h0 = relu(x @ W1.T + b1); table entries for l=0
            XG = 2
            with tc.tile_pool(name="xpool", bufs=2) as xpool:
                for g0 in range(0, TPC, XG):
                    g1 = min(g0 + XG, TPC)
                    cw = (g1 - g0) * 128
                    xt = xpool.tile([128, NSLC, cw], BF16, tag="xt")
                    nc.sync.dma_start(
                        xt[:, :, :],
                        xT_h[:, g0 * 128:g1 * 128].rearrange(
                            "(s p) c -> p s c", p=128))
                    for t in range(g0, g1):
                        lc = (t - g0) * 128
                        acc = apsum.tile([128, H], F32, tag="acc")
                        nc.tensor.matmul(acc[:], lhsT=onesb[:], rhs=b1s[:],
                                         start=True, stop=False)
                        for s in range(NSLC):
                            nc.tensor.matmul(acc[:], lhsT=xt[:, s, lc:lc + 128],
                                             rhs=W1Ts[:, s, :],
                                             start=False, stop=(s == NSLC - 1))
                        nc.scalar.activation(stage[:, t, :], acc[:], AF.Relu)
                        nc.vector.tensor_copy(raw[:, t, :], stage[:, t, :])
                        table_entries(t, 0)

            # ---- layers
            qload = [0] * cfg.NQ  # greedy per-queue index balance
            issue_tbl(0)
            for l in range(L):
                tbl_full = tbl_fulls.pop(l)

                for (ct0, ct1) in chunks:
                    spans = []
                    gts = []
                    for w in range(NW):
                        c0, c1 = int(offs[w][ct0]), int(offs[w][ct1])
                        spans.append((c0, c1))
                        if c1 == c0:
                            gts.append(None)
                            continue
                        gw = gpool.tile([128, cfg.CHUNK_COLS, 128], BF16,
                                        tag=f"g{w}")
                        lo = cfg.wbases[w]
                        q = min(range(cfg.NQ), key=lambda i: qload[i])
                        qload[q] += c1 - c0
                        kw = dict(
                            out_ap=gw[:, :c1 - c0, :],
                            in_ap=tbl_full[lo:lo + cfg.WINDOW, :],
                            idxs_ap=idxs[w][:, 8 * c0:8 * c1],
                            num_idxs=128 * (c1 - c0),
                            num_idxs_reg=128 * (c1 - c0),
                            elem_size=128, single_packet=False,
                            queue_num=q)
                        if use_prepare:
                            nc.gpsimd.dma_gather(prepare_only=True,
                                                 sem=dma_sem, **kw)
                        else:
                            nc.gpsimd.dma_gather(**kw)
                        gts.append(gw)
                    if use_prepare:
                        nc.gpsimd.trigger_dma(count=None)

                    # coefficients + messages per window
                    msgs = []
                    for w in range(NW):
                        c0, c1 = spans[w]
                        if c1 == c0:
                            msgs.append(None)
                            continue
                        gw = gts[w]
                        span = c1 - c0
                        cf = cpool.tile([128, cfg.CHUNK_COLS], BF16, tag=f"cf{w}")
                        gw_f32 = gw[:, :span, :].bitcast(F32)
                        for t in range(ct0, ct1):
                            nn = int(caps[w][t])
                            if nn == 0:
                                continue
                            lc = int(offs[w][t]) - c0
                            al_src = (gw_f32[:, lc:lc + nn, 32:33] if al_f32
                                      else gw[:, lc:lc + nn, 64:65])
                            nc.scalar.activation(
                                cf[:, lc:lc + nn], al_src,
                                AF.Tanh, bias=ar_sb[:, t:t + 1])
                        msg = cpool.tile([128, cfg.CHUNK_COLS, H], BF16,
                                         tag=f"msg{w}")
                        nc.vector.tensor_tensor(
                            out=msg[:, :span, :],
                            in0=gw[:, :span, 0:H],
                            in1=cf[:, :span].to_broadcast((128, span, H)),
                            op=OP.mult)
                        msgs.append(msg)

                    for t in range(ct0, ct1):
                        nblk = sum(int(caps[w][t]) for w in range(NW))
                        acc = apsum.tile([128, H], F32, tag="acc")
                        if self_local:
                            # self message: selfw*dinv*tanh(al+ar)*h, from the
                            # local table row (tstage = dinv*h, al)
                            cfs = cpool.tile([128, 1], F32, tag="cfs")
                            al_self = (tstage_f32[:, t, 32:33] if al_f32
                                       else tstage[:, t, 64:65])
                            nc.scalar.activation(cfs[:], al_self, AF.Tanh,
                                                 bias=ar_sb[:, t:t + 1])
                            msl = cpool.tile([128, H], BF16, tag="msgself")
                            nc.vector.tensor_scalar(
                                out=msl[:], in0=tstage[:, t, 0:H],
                                scalar1=cfs[:, 0:1],
                                scalar2=selfw[:, t:t + 1],
                                op0=OP.mult, op1=OP.mult)
                            nc.tensor.matmul(acc[:], lhsT=identb[:],
                                             rhs=msl[:], start=True,
                                             stop=(nblk == 0))
                            bi = 1
                            nblk += 1
                        else:
                            bi = 0
                        for w in range(NW):
                            nn = int(caps[w][t])
                            if nn == 0:
                                continue
                            lc = int(offs[w][t]) - spans[w][0]
                            for b in range(nn):
                                nc.tensor.matmul(
                                    acc[:], lhsT=identb[:],
                                    rhs=msgs[w][:, lc + b, :],
                                    start=(bi == 0), stop=(bi == nblk - 1))
                                bi += 1
                        tmp = cpool.tile([128, H], F32, tag="tmp")
                        nc.scalar.activation(
                            tmp[:], acc[:], AF.Copy,
                            scale=dinv[:, t:t + 1])
                        nc.vector.scalar_tensor_tensor(
                            out=stage[:, t, :], in0=raw[:, t, :],
                            scalar=cfg.EPS, in1=tmp[:],
                            op0=OP.mult, op1=OP.add)
                        if l < L - 1:
                            table_entries(t, l + 1)
                        else:
                            # logits for tile t, interleaved into the last
                            # layer's chunk processing
                            tr = bpsum.tile([H, 128], F32, tag="tr")
                            nc.tensor.transpose(out=tr[:], in_=stage[:, t, :],
                                                identity=ident[:])
                            htT = spool.tile([H, 128], F32, tag="htT")
                            nc.vector.tensor_copy(htT[:], tr[:])
                            lg = bpsum.tile([128, C], F32, tag="lg")
                            nc.tensor.matmul(lg[:], lhsT=onesf[:], rhs=b2s[:],
                                             start=True, stop=False)
                            nc.tensor.matmul(lg[:], lhsT=htT[:], rhs=W2Ts[:],
                                             start=False, stop=True)
                            nc.vector.tensor_reduce(
                                out=mx_all[:, t:t + 1], in_=lg[:],
                                axis=mybir.AxisListType.X, op=OP.max,
                                negate=True)
                            scr40 = cpool.tile([128, C], F32, tag="scr40")
                            nc.scalar.activation(scr40[:], lg[:], AF.Exp,
                                                 bias=mx_all[:, t:t + 1],
                                                 accum_out=se_all[:, t:t + 1])
                            nc.vector.tensor_copy(outs[:, t, :], lg[:])
                if l < L - 1:
                    issue_tbl(l + 1)

            # ---- log_softmax epilogue
            nc.scalar.activation(lse_all[:], se_all[:], AF.Ln)
            for t in range(TPC):
                nc.vector.tensor_scalar(
                    out=outs[:, t, :], in0=outs[:, t, :],
                    scalar1=mx_all[:, t:t + 1], scalar2=lse_all[:, t:t + 1],
                    op0=OP.add, op1=OP.subtract)
            nc.sync.dma_start(out_h[:].rearrange("(t p) c -> p t c", p=128),
                              outs[:])
    nc.compile()
    return nc


def run(cfg: Cfg, inputs: dict, trace: bool = False, use_prepare: bool = None):
    if use_prepare is None:
        import os
        use_prepare = os.environ.get("KPREP", "0") == "1"
    in_maps, orders, CAPS = host_prep(cfg, **inputs)
    nc = build_nc(cfg, CAPS, use_prepare=use_prepare)
    res = bass_utils.run_bass_kernel_spmd(
        nc, in_maps, core_ids=list(range(cfg.M)), trace=trace)
    out = np.empty((cfg.N, cfg.C), dtype=np.float32)
    rop = cfg.rank_of_pos
    for k in range(cfg.M):
        out[k * cfg.NSH + orders[k]] = np.asarray(res.results[k]["out"],
                                                  np.float32)[rop]
    return out, res


def kernel(x, edge_index, W1, b1, W2, b2, att_l, att_r):
    cfg = Cfg()
    out, _ = run(cfg, dict(x=np.asarray(x, np.float32),
                           edge_index=np.asarray(edge_index),
                           W1=W1, b1=b1, W2=W2, b2=b2,
                           att_l=att_l, att_r=att_r))
    return out

